# revision 13
# baseline (speedup 1.0000x reference)
"""Bass/Trainium2 kernel for nn_DiffusionTransformer (4-block diffusion transformer).

Sharding: 8 cores = 2 (batch) x 4 (query-row chunks of 96).
Per layer, each core computes q/k/v/g for its own 96 rows; k/v are
all-gathered within the 4-core batch group; attention + transition are
row-sharded.  The pair bias z @ wb (+bb) is precomputed once for all 4
layers from a host-pretransposed z shard.  All matmuls run in bf16 with
f32 accumulation; layernorms / softmax / residual stream are f32.
"""

import numpy as np
import ml_dtypes

BF = ml_dtypes.bfloat16

B, N, D, H, DH, DP, DS, NT, L = 2, 384, 768, 16, 48, 128, 384, 2, 4
DHP = 64                      # padded head dim (partition alignment)
HP = H * DHP                  # 1024
G = 4                         # cores per batch
CH = N // G                   # 96 rows per core
KT_SZ = HP * CH               # 98304  (kT block)
V_SZ = CH * D                 # 73728  (v block)
EPS = 1e-5
NCORES = 8

_CACHE = {}


def _f32(x):
    return np.ascontiguousarray(np.asarray(x, dtype=np.float32))


def _bf(x):
    return np.ascontiguousarray(np.asarray(x, dtype=np.float32).astype(BF))


# ---------------------------------------------------------------------------
# device program
# ---------------------------------------------------------------------------

def _build_nc(debug=False):
    from concourse import bacc
    import concourse.mybir as mybir
    from concourse.tile import TileContext
    from concourse.masks import make_identity

    f32 = mybir.dt.float32
    bf = mybir.dt.bfloat16
    ALU = mybir.AluOpType
    ACTF = mybir.ActivationFunctionType
    AX = mybir.AxisListType

    nc = bacc.Bacc(num_devices=NCORES)

    # ---- kernel I/O -------------------------------------------------------
    t_a = nc.dram_tensor("a0", [CH, D], f32, kind="ExternalInput")
    t_s = nc.dram_tensor("s0", [CH, DS], f32, kind="ExternalInput")
    t_beta = nc.dram_tensor("beta0", [CH, N], f32, kind="ExternalInput")
    t_zt = nc.dram_tensor("zt", [DP, CH, N], bf, kind="ExternalInput")
    t_wb = nc.dram_tensor("wb_all", [DP, 64], bf, kind="ExternalInput")
    t_bb = nc.dram_tensor("bb_all", [64, 1], f32, kind="ExternalInput")
    t_wq, t_wk, t_wv, t_wg, t_wo, t_w1, t_w2, t_w3, t_bq = [], [], [], [], [], [], [], [], []
    t_sw, t_sbr = [], []
    for l in range(L):
        t_wq.append(nc.dram_tensor(f"wq{l}", [D, D], bf, kind="ExternalInput"))
        t_wk.append(nc.dram_tensor(f"wk{l}", [D, D], bf, kind="ExternalInput"))
        t_wv.append(nc.dram_tensor(f"wv{l}", [D, D], bf, kind="ExternalInput"))
        t_wg.append(nc.dram_tensor(f"wg{l}", [D, D], bf, kind="ExternalInput"))
        t_wo.append(nc.dram_tensor(f"wo{l}", [HP, D], bf, kind="ExternalInput"))
        t_w1.append(nc.dram_tensor(f"w1{l}", [D, NT * D], bf, kind="ExternalInput"))
        t_w2.append(nc.dram_tensor(f"w2{l}", [D, NT * D], bf, kind="ExternalInput"))
        t_w3.append(nc.dram_tensor(f"w3{l}", [NT * D, D], bf, kind="ExternalInput"))
        t_bq.append(nc.dram_tensor(f"bq{l}", [128, 8], f32, kind="ExternalInput"))
        t_sw.append([nc.dram_tensor(f"sw{l}_{j}", [DS, D], bf, kind="ExternalInput")
                     for j in range(6)])
        t_sbr.append([nc.dram_tensor(f"sbr{l}_{j}", [1, D], bf, kind="ExternalInput")
                      for j in range(6)])
    t_out = nc.dram_tensor("out", [CH, D], f32, kind="ExternalOutput")
    t_dbg = {}
    if debug:
        for nm, shp in [("d_sg", [6 * CH, D]), ("d_bias", [16 * CH, N]),
                        ("d_a2", [CH, D]), ("d_qrm", [CH, D]),
                        ("d_kTf", [128, 8 * G * CH]), ("d_lg", [CH, N]),
                        ("d_en", [CH, N]), ("d_opre", [128, 8 * CH]),
                        ("d_a1", [CH, D]), ("d_tr", [CH, D]),
                        ("d_attg", [CH, D]), ("d_a2t", [CH, D]),
                        ("d_hh", [CH, NT * D])]:
            t_dbg[nm] = nc.dram_tensor(nm, shp, f32, kind="ExternalOutput")

    # ---- internal DRAM ----------------------------------------------------
    kv_in = nc.dram_tensor("kv_in", [KT_SZ + V_SZ], bf, kind="Internal")
    kv_out = nc.dram_tensor("kv_out", [G, KT_SZ + V_SZ], bf, kind="Internal")
    bias_sc = nc.dram_tensor("bias_sc", [64, CH, N], bf, kind="Internal")
    sg_dram = nc.dram_tensor("sg_dram", [L * 6, CH, D], bf, kind="Internal")

    RG = [[0, 1, 2, 3], [4, 5, 6, 7]]

    with TileContext(nc) as tc:
        import contextlib
        ctx = contextlib.ExitStack()
        with ctx:
            cpool = ctx.enter_context(tc.tile_pool(name="const", bufs=1))
            ppool = ctx.enter_context(tc.tile_pool(name="pers", bufs=1))
            wqk = ctx.enter_context(tc.tile_pool(name="wqk", bufs=1))
            wtr = ctx.enter_context(tc.tile_pool(name="wtr", bufs=1))
            wop = ctx.enter_context(tc.tile_pool(name="wop", bufs=1))
            swp = ctx.enter_context(tc.tile_pool(name="swp", bufs=1))
            sgp = ctx.enter_context(tc.tile_pool(name="sgp", bufs=1))
            kvp = ctx.enter_context(tc.tile_pool(name="kvp", bufs=1))
            work = ctx.enter_context(tc.tile_pool(name="work", bufs=1))
            hwork = ctx.enter_context(tc.tile_pool(name="hwork", bufs=1))
            small = ctx.enter_context(tc.tile_pool(name="small", bufs=4))
            bld = ctx.enter_context(tc.tile_pool(name="bld", bufs=3))
            ztl = ctx.enter_context(tc.tile_pool(name="ztl", bufs=4))
            pmm = ctx.enter_context(tc.tile_pool(name="pmm", bufs=3, space="PSUM"))
            ptp = ctx.enter_context(tc.tile_pool(name="ptp", bufs=3, space="PSUM"))
            pav = ctx.enter_context(tc.tile_pool(name="pav", bufs=1, space="PSUM"))

            # ---- constants / persistents ----------------------------------
            ident = cpool.tile([128, 128], bf, tag="ident")
            make_identity(nc, ident[:, :])
            ones1 = cpool.tile([1, CH], bf, tag="ones1")
            nc.vector.memset(ones1[:, :], 1.0)
            beta_sb = cpool.tile([CH, N], f32, tag="beta")
            nc.sync.dma_start(out=beta_sb[:, :], in_=t_beta.ap())
            bb_sb = cpool.tile([64, 1], f32, tag="bb")
            nc.sync.dma_start(out=bb_sb[:, :], in_=t_bb.ap())
            wb_sb = cpool.tile([DP, 64], bf, tag="wb")
            nc.sync.dma_start(out=wb_sb[:, :], in_=t_wb.ap())

            a_bufs = [ppool.tile([CH, D], f32, tag="abuf0", name="abuf0"),
                      ppool.tile([CH, D], f32, tag="abuf1", name="abuf1")]
            nc.sync.dma_start(out=a_bufs[0][:, :], in_=t_a.ap())
            o_preT = ppool.tile([128, 8, CH], bf, tag="opreT")
            nc.vector.memset(o_preT[:, :, :], 0.0)

            # ---- helpers ---------------------------------------------------
            def transpose_to(dst_ap, src_ap, pc, fc, add_scalar=None):
                """dst[fc,pc] = src[pc,fc].T  (bf16 via PE transpose)."""
                pt = ptp.tile([128, 128], bf, tag="pt")
                nc.tensor.transpose(pt[:fc, :pc], src_ap, ident[:pc, :pc])
                if add_scalar is None:
                    nc.vector.tensor_copy(out=dst_ap, in_=pt[:fc, :pc])
                else:
                    nc.vector.tensor_scalar_add(dst_ap, pt[:fc, :pc], add_scalar)

            def ln_stats(x_ap, dfree):
                """row layernorm stats of f32 [CH, dfree] -> (negmean, rstd)."""
                sm = small.tile([CH, 1], f32, tag="sm")
                nc.vector.reduce_sum(out=sm[:, :], in_=x_ap, axis=AX.X)
                sqs = work.tile([CH, D], bf, tag="sqscr")
                ssq = small.tile([CH, 1], f32, tag="ssq")
                nc.scalar.activation(sqs[:, :dfree], x_ap, ACTF.Square,
                                     accum_out=ssq[:, :])
                negm = small.tile([CH, 1], f32, tag="negm")
                nc.vector.tensor_scalar_mul(negm[:, :], sm[:, :], -1.0 / dfree)
                v1 = small.tile([CH, 1], f32, tag="v1")
                nc.vector.tensor_scalar(v1[:, :], ssq[:, :], 1.0 / dfree, EPS,
                                        ALU.mult, ALU.add)
                m2 = small.tile([CH, 1], f32, tag="m2")
                nc.vector.tensor_tensor(out=m2[:, :], in0=negm[:, :],
                                        in1=negm[:, :], op=ALU.mult)
                v2 = small.tile([CH, 1], f32, tag="v2")
                nc.vector.tensor_tensor(out=v2[:, :], in0=v1[:, :], in1=m2[:, :],
                                        op=ALU.subtract)
                sd = small.tile([CH, 1], f32, tag="sd")
                nc.scalar.sqrt(sd[:, :], v2[:, :])
                rstd = small.tile([CH, 1], f32, tag="rstd")
                nc.vector.reciprocal(rstd[:, :], sd[:, :])
                return negm, rstd

            # ---- phase 1: s-dependent gates (all layers) -------------------
            s_sb = work.tile([CH, DS], f32, tag="s_sb")
            nc.sync.dma_start(out=s_sb[:, :], in_=t_s.ap())
            negm, rstd = ln_stats(s_sb[:, :], DS)
            ns_bf = ppool.tile([CH, DS], bf, tag="ns_bf")
            nc.vector.tensor_scalar(ns_bf[:, :], s_sb[:, :], negm[:, :],
                                    rstd[:, :], ALU.add, ALU.mult)
            s_bf = ppool.tile([CH, DS], bf, tag="s_bf")
            nc.vector.tensor_copy(out=s_bf[:, :], in_=s_sb[:, :])
            nsT = ppool.tile([128, 3, CH], bf, tag="nsT")
            sT = ppool.tile([128, 3, CH], bf, tag="sT")
            for c in range(3):
                transpose_to(nsT[:, c, :], ns_bf[:, c * 128:(c + 1) * 128], CH, 128)
                transpose_to(sT[:, c, :], s_bf[:, c * 128:(c + 1) * 128], CH, 128)

            for l in range(L):
                for j in range(6):
                    wmat = swp.tile([128, 3, D], bf, tag="swmat")
                    nc.sync.dma_start(
                        out=wmat[:, :, :],
                        in_=t_sw[l][j].ap().rearrange("(kc p) n -> p kc n", p=128))
                    brow = swp.tile([1, D], bf, tag="brow")
                    nc.sync.dma_start(out=brow[:, :], in_=t_sbr[l][j].ap())
                    lhs = nsT if j in (0, 1, 3, 4) else sT
                    og = work.tile([CH, D], bf, tag="sgout")
                    for hf in range(2):
                        ps = pmm.tile([CH, 512], f32, tag="pmm")
                        sl = slice(hf * 384, (hf + 1) * 384)
                        for kc in range(3):
                            nc.tensor.matmul(ps[:, :384], lhs[:, kc, :],
                                             wmat[:, kc, sl],
                                             start=(kc == 0), stop=False)
                        nc.tensor.matmul(ps[:, :384], ones1[:, :], brow[:, sl],
                                         start=False, stop=True)
                        if j in (0, 2, 3, 5):
                            nc.scalar.activation(og[:, sl], ps[:, :384], ACTF.Sigmoid)
                        else:
                            nc.vector.tensor_copy(out=og[:, sl], in_=ps[:, :384])
                    nc.sync.dma_start(out=sg_dram.ap()[l * 6 + j], in_=og[:, :])

            # ---- phase 2: pair bias for all layers -------------------------
            for i in range(CH):
                zt = ztl.tile([DP, N], bf, tag="zt")
                nc.sync.dma_start(out=zt[:, :], in_=t_zt.ap()[:, i, :])
                ps = pav.tile([64, N], f32, tag="pbias")
                nc.tensor.matmul(ps[:, :], wb_sb[:, :], zt[:, :],
                                 start=True, stop=True)
                ob = bld.tile([64, N], bf, tag="bwr")
                nc.vector.tensor_scalar_add(ob[:, :], ps[:, :], bb_sb[:, :])
                nc.sync.dma_start(out=bias_sc.ap()[:, i, :], in_=ob[:, :])

            # ---- phase 3: transformer layers -------------------------------
            def adaln(a_ap, sg_sb, base):
                negm, rstd = ln_stats(a_ap, D)
                an = work.tile([CH, D], bf, tag="an")
                nc.vector.tensor_scalar(an[:, :], a_ap, negm[:, :], rstd[:, :],
                                        ALU.add, ALU.mult)
                t1 = work.tile([CH, D], bf, tag="adtmp")
                nc.vector.tensor_tensor(out=t1[:, :], in0=an[:, :],
                                        in1=sg_sb[:, base, :], op=ALU.mult)
                a2 = work.tile([CH, D], bf, tag="a2_" + str(base))
                nc.vector.tensor_tensor(out=a2[:, :], in0=t1[:, :],
                                        in1=sg_sb[:, base + 1, :], op=ALU.add)
                a2T = work.tile([128, 6, CH], bf, tag="a2T_" + str(base))
                for c in range(6):
                    transpose_to(a2T[:, c, :], a2[:, c * 128:(c + 1) * 128], CH, 128)
                return a2T, a2

            for l in range(L):
                ain = a_bufs[l % 2]
                aout = a_bufs[(l + 1) % 2]

                sg_sb = sgp.tile([CH, 6, D], bf, tag="sg")
                nc.sync.dma_start(
                    out=sg_sb[:, :, :],
                    in_=sg_dram.ap()[l * 6:l * 6 + 6].rearrange("t p n -> p t n"))

                a2T, a2dbg = adaln(ain[:, :], sg_sb, 0)
                if debug and l == 0:
                    nc.gpsimd.dma_start(out=t_dbg["d_a2"].ap(), in_=a2dbg[:, :])

                # qkvg (row-major, 96 own rows)
                rms = {}
                for nm, wd in (("q", t_wq[l]), ("k", t_wk[l]), ("v", t_wv[l]),
                               ("g", t_wg[l])):
                    wsb = wqk.tile([128, 6, D], bf, tag="w_" + nm)
                    nc.sync.dma_start(
                        out=wsb[:, :, :],
                        in_=wd.ap().rearrange("(kc p) n -> p kc n", p=128))
                    rmt = work.tile([CH, D], bf, tag="rm_" + nm)
                    for hf in range(2):
                        ps = pmm.tile([CH, 512], f32, tag="pmm")
                        sl = slice(hf * 384, (hf + 1) * 384)
                        for kc in range(6):
                            nc.tensor.matmul(ps[:, :384], a2T[:, kc, :],
                                             wsb[:, kc, sl],
                                             start=(kc == 0), stop=(kc == 5))
                        nc.vector.tensor_copy(out=rmt[:, sl], in_=ps[:, :384])
                    rms[nm] = rmt
                    if debug and l == 0 and nm == "q":
                        nc.gpsimd.dma_start(out=t_dbg["d_qrm"].ap(), in_=rmt[:, :])

                # local kT -> kv_in ; v -> kv_in ; AllGather
                kT_loc = work.tile([128, 8, CH], bf, tag="kTl")
                for h in range(H):
                    transpose_to(kT_loc[64 * (h % 2):64 * (h % 2) + DH, h // 2, :],
                                 rms["k"][:, DH * h:DH * (h + 1)], CH, DH)
                nc.sync.dma_start(
                    out=kv_in.ap()[0:KT_SZ].rearrange("(fc p j) -> p fc j",
                                                      p=128, j=CH),
                    in_=kT_loc[:, :, :])
                nc.sync.dma_start(
                    out=kv_in.ap()[KT_SZ:].rearrange("(p n) -> p n", p=CH),
                    in_=rms["v"][:, :])
                nc.gpsimd.collective_compute(
                    "AllGather", ALU.bypass, replica_groups=RG,
                    ins=[kv_in.ap()], outs=[kv_out.ap()])

                # qT (+bias), gT
                bq_sb = small.tile([128, 8], f32, tag="bq")
                nc.sync.dma_start(out=bq_sb[:, :], in_=t_bq[l].ap())
                qT = work.tile([128, 8, CH], bf, tag="qT")
                gT = work.tile([128, 8, CH], bf, tag="gT")
                for h in range(H):
                    po = 64 * (h % 2)
                    transpose_to(qT[po:po + DH, h // 2, :],
                                 rms["q"][:, DH * h:DH * (h + 1)], CH, DH,
                                 add_scalar=bq_sb[po:po + DH, h // 2:h // 2 + 1])
                    transpose_to(gT[po:po + DH, h // 2, :],
                                 rms["g"][:, DH * h:DH * (h + 1)], CH, DH)

                # ---- transition (overlaps the AllGather) ----
                a2tT, a2t_dbg = adaln(ain[:, :], sg_sb, 3)
                if debug and l == 0:
                    nc.gpsimd.dma_start(out=t_dbg["d_a2t"].ap(), in_=a2t_dbg[:, :])
                w1_sb = wtr.tile([128, 6, NT * D], bf, tag="w1")
                nc.sync.dma_start(
                    out=w1_sb[:, :, :],
                    in_=t_w1[l].ap().rearrange("(kc p) n -> p kc n", p=128))
                w2_sb = wtr.tile([128, 6, NT * D], bf, tag="w2")
                nc.sync.dma_start(
                    out=w2_sb[:, :, :],
                    in_=t_w2[l].ap().rearrange("(kc p) n -> p kc n", p=128))
                w3_sb = wtr.tile([128, 12, D], bf, tag="w3")
                nc.sync.dma_start(
                    out=w3_sb[:, :, :],
                    in_=t_w3[l].ap().rearrange("(kc p) n -> p kc n", p=128))
                h1 = hwork.tile([CH, NT * D], bf, tag="h1")
                h2 = hwork.tile([CH, NT * D], bf, tag="h2")
                for hf in range(3):
                    sl = slice(hf * 512, (hf + 1) * 512)
                    ps1 = pmm.tile([CH, 512], f32, tag="pmm")
                    for kc in range(6):
                        nc.tensor.matmul(ps1[:, :], a2tT[:, kc, :],
                                         w1_sb[:, kc, sl],
                                         start=(kc == 0), stop=(kc == 5))
                    nc.scalar.activation(h1[:, sl], ps1[:, :], ACTF.Silu)
                    ps2 = pmm.tile([CH, 512], f32, tag="pmm")
                    for kc in range(6):
                        nc.tensor.matmul(ps2[:, :], a2tT[:, kc, :],
                                         w2_sb[:, kc, sl],
                                         start=(kc == 0), stop=(kc == 5))
                    nc.vector.tensor_copy(out=h2[:, sl], in_=ps2[:, :])
                hh = hwork.tile([CH, NT * D], bf, tag="hh")
                nc.vector.tensor_tensor(out=hh[:, :], in0=h1[:, :], in1=h2[:, :],
                                        op=ALU.mult)
                if debug and l == 0:
                    nc.gpsimd.dma_start(out=t_dbg["d_hh"].ap(), in_=hh[:, :])
                hhT = hwork.tile([128, 12, CH], bf, tag="hhT")
                for c in range(12):
                    transpose_to(hhT[:, c, :], hh[:, c * 128:(c + 1) * 128], CH, 128)
                for hf in range(2):
                    sl = slice(hf * 384, (hf + 1) * 384)
                    ps = pmm.tile([CH, 512], f32, tag="pmm")
                    for kc in range(12):
                        nc.tensor.matmul(ps[:, :384], hhT[:, kc, :],
                                         w3_sb[:, kc, sl],
                                         start=(kc == 0), stop=(kc == 11))
                    # overwrite aout with gated transition output
                    nc.vector.tensor_tensor(out=aout[:, sl], in0=ps[:, :384],
                                            in1=sg_sb[:, 5, sl], op=ALU.mult)

                if debug and l == 0:
                    nc.gpsimd.dma_start(out=t_dbg["d_tr"].ap(), in_=aout[:, :])
                # ---- attention (needs the AllGather) ----
                kTf = kvp.tile([128, 8, G, CH], bf, tag="kTf")
                for c in range(G):
                    nc.sync.dma_start(
                        out=kTf[:, :, c, :],
                        in_=kv_out.ap()[c, 0:KT_SZ].rearrange(
                            "(fc p j) -> p fc j", p=128, j=CH))
                if debug and l == 0:
                    nc.gpsimd.dma_start(
                        out=t_dbg["d_kTf"].ap().rearrange(
                            "p (a c j) -> p a c j", a=8, c=G),
                        in_=kTf[:, :, :, :])
                vf = kvp.tile([CH, G, D], bf, tag="vf")
                nc.sync.dma_start(
                    out=vf[:, :, :],
                    in_=kv_out.ap()[:, KT_SZ:].rearrange("c (p n) -> p c n", p=CH))

                for h in range(H):
                    po = 64 * (h % 2)
                    hc = h // 2
                    bt = bld.tile([CH, N], bf, tag="bt")
                    nc.sync.dma_start(out=bt[:, :], in_=bias_sc.ap()[l * 16 + h])
                    pl = pmm.tile([CH, 512], f32, tag="pmm")
                    nc.tensor.matmul(pl[:, :N], qT[po:po + DH, hc, :],
                                     kTf[po:po + DH, hc, :, :],
                                     start=True, stop=True)
                    lg = work.tile([CH, N], f32, tag="lg")
                    nc.vector.tensor_tensor(out=lg[:, :], in0=pl[:, :N],
                                            in1=bt[:, :], op=ALU.add)
                    lg2 = work.tile([CH, N], f32, tag="lg2")
                    nc.vector.tensor_tensor(out=lg2[:, :], in0=lg[:, :],
                                            in1=beta_sb[:, :], op=ALU.add)
                    if debug and l == 0 and h == 0:
                        nc.sync.dma_start(out=t_dbg["d_lg"].ap(), in_=lg2[:, :])
                    mx = small.tile([CH, 1], f32, tag="mx")
                    nc.vector.reduce_max(out=mx[:, :], in_=lg2[:, :], axis=AX.X)
                    nmx = small.tile([CH, 1], f32, tag="nmx")
                    nc.vector.tensor_scalar_mul(nmx[:, :], mx[:, :], -1.0)
                    ex = work.tile([CH, N], bf, tag="ex")
                    se = small.tile([CH, 1], f32, tag="se")
                    nc.scalar.activation(ex[:, :], lg2[:, :], ACTF.Exp,
                                         bias=nmx[:, :], accum_out=se[:, :])
                    rs = small.tile([CH, 1], f32, tag="rs")
                    nc.vector.reciprocal(rs[:, :], se[:, :])
                    en = work.tile([CH, N], bf, tag="en")
                    nc.vector.tensor_scalar_mul(en[:, :], ex[:, :], rs[:, :])
                    if debug and l == 0 and h == 0:
                        nc.gpsimd.dma_start(out=t_dbg["d_en"].ap(), in_=en[:, :])
                    eT = work.tile([CH, G, CH], bf, tag="eT")
                    for c in range(G):
                        transpose_to(eT[:, c, :], en[:, c * CH:(c + 1) * CH], CH, CH)
                    pa = pav.tile([DH, CH], f32, tag="pattv")
                    for c in range(G):
                        nc.tensor.matmul(pa[:, :], vf[:, c, DH * h:DH * (h + 1)],
                                         eT[:, c, :],
                                         start=(c == 0), stop=(c == G - 1))
                    sgm = small.tile([DH, CH], bf, tag="sgm")
                    nc.scalar.activation(sgm[:, :], gT[po:po + DH, hc, :],
                                         ACTF.Sigmoid)
                    nc.vector.tensor_tensor(out=o_preT[po:po + DH, hc, :],
                                            in0=pa[:, :], in1=sgm[:, :],
                                            op=ALU.mult)

                if debug and l == 0:
                    nc.gpsimd.dma_start(
                        out=t_dbg["d_opre"].ap().rearrange("p (a j) -> p a j", a=8),
                        in_=o_preT[:, :, :])
                wo_sb = wop.tile([128, 8, D], bf, tag="wo")
                nc.sync.dma_start(
                    out=wo_sb[:, :, :],
                    in_=t_wo[l].ap().rearrange("(kc p) n -> p kc n", p=128))
                for hf in range(2):
                    sl = slice(hf * 384, (hf + 1) * 384)
                    ps = pmm.tile([CH, 512], f32, tag="pmm")
                    for kc in range(8):
                        nc.tensor.matmul(ps[:, :384], o_preT[:, kc, :],
                                         wo_sb[:, kc, sl],
                                         start=(kc == 0), stop=(kc == 7))
                    gat = work.tile([CH, 384], f32, tag="attg")
                    nc.vector.tensor_tensor(out=gat[:, :], in0=ps[:, :384],
                                            in1=sg_sb[:, 2, sl], op=ALU.mult)
                    if debug and l == 0:
                        nc.gpsimd.dma_start(
                            out=t_dbg["d_attg"].ap()[:, sl], in_=gat[:, :])
                    nc.vector.tensor_tensor(out=aout[:, sl], in0=aout[:, sl],
                                            in1=gat[:, :], op=ALU.add)

                if debug and l == 0:
                    nc.gpsimd.dma_start(out=t_dbg["d_a1"].ap(), in_=aout[:, :])

            if debug:
                nc.gpsimd.dma_start(
                    out=t_dbg["d_sg"].ap().rearrange("(t p) n -> p t n", p=CH),
                    in_=sg_dram.ap()[0:6].rearrange("t p n -> p t n"))
                nc.gpsimd.dma_start(
                    out=t_dbg["d_bias"].ap().rearrange("(t p) n -> p t n", p=CH),
                    in_=bias_sc.ap()[0:16].rearrange("t p n -> p t n"))
            nc.sync.dma_start(out=t_out.ap(), in_=a_bufs[L % 2][:, :])

    nc.finalize()
    return nc


# ---------------------------------------------------------------------------
# host side
# ---------------------------------------------------------------------------

def _prep_shared(params):
    """Layer weights, preprocessed + cast.  Shared across all cores."""
    P = {}
    scale = DH ** -0.5
    wb_cols, bb_cols = [], []
    for l, p in enumerate(params):
        att, tr = p["att"], p["trans"]
        ad = att["adaln"]
        ln_g, ln_b = _f32(ad["ln_g"]), _f32(ad["ln_b"])
        w_s, w_s_b, b_s = _f32(ad["w_s"]), _f32(ad["w_s_b"]), _f32(ad["b_s"])
        P[f"sw{l}_0"] = _bf(ln_g[:, None] * w_s)
        P[f"sbr{l}_0"] = _bf((w_s_b + ln_b @ w_s)[None, :])
        P[f"sw{l}_1"] = _bf(ln_g[:, None] * b_s)
        P[f"sbr{l}_1"] = _bf((ln_b @ b_s)[None, :])
        P[f"sw{l}_2"] = _bf(att["ws"])
        P[f"sbr{l}_2"] = _bf(_f32(att["bs"])[None, :])
        ad2 = tr["adaln"]
        ln_g2, ln_b2 = _f32(ad2["ln_g"]), _f32(ad2["ln_b"])
        w_s2, w_s_b2, b_s2 = _f32(ad2["w_s"]), _f32(ad2["w_s_b"]), _f32(ad2["b_s"])
        P[f"sw{l}_3"] = _bf(ln_g2[:, None] * w_s2)
        P[f"sbr{l}_3"] = _bf((w_s_b2 + ln_b2 @ w_s2)[None, :])
        P[f"sw{l}_4"] = _bf(ln_g2[:, None] * b_s2)
        P[f"sbr{l}_4"] = _bf((ln_b2 @ b_s2)[None, :])
        P[f"sw{l}_5"] = _bf(tr["wg"])
        P[f"sbr{l}_5"] = _bf(_f32(tr["bg"])[None, :])

        P[f"wq{l}"] = _bf(_f32(att["wq"]) * scale)
        P[f"wk{l}"] = _bf(att["wk"])
        P[f"wv{l}"] = _bf(att["wv"])
        P[f"wg{l}"] = _bf(att["wg"])
        bq = _f32(att["bq"]) * scale
        bqp = np.zeros(HP, np.float32)
        bqp.reshape(H, DHP)[:, :DH] = bq.reshape(H, DH)
        P[f"bq{l}"] = _f32(bqp.reshape(8, 128).T)
        wo = _f32(att["wo"])
        wop_ = np.zeros((HP, D), np.float32)
        wop_.reshape(H, DHP, D)[:, :DH] = wo.reshape(H, DH, D)
        P[f"wo{l}"] = _bf(wop_)
        P[f"w1{l}"] = _bf(tr["w1"])
        P[f"w2{l}"] = _bf(tr["w2"])
        P[f"w3{l}"] = _bf(tr["w3"])
        wb_cols.append(_f32(att["wb"]))
        bb_cols.append(_f32(att["bb"]))
    P["wb_all"] = _bf(np.concatenate(wb_cols, axis=1))          # [128, 64]
    P["bb_all"] = _f32(np.concatenate(bb_cols).reshape(64, 1))  # hl = l*16+h
    return P


def kernel(a, s, z, beta, params):
    from concourse.bass_utils import run_bass_kernel_spmd
    import os

    a, s, beta = _f32(a), _f32(s), _f32(beta)
    z = np.asarray(z, dtype=np.float32)

    dbg = bool(int(os.environ.get("KERNEL_DEBUG", "0")))
    key = ("nc", dbg)
    if key not in _CACHE:
        _CACHE[key] = _build_nc(debug=dbg)
    nc = _CACHE[key]

    shared = _prep_shared(params)
    in_maps = []
    for core in range(NCORES):
        b, g = divmod(core, G)
        i0 = g * CH
        m = dict(shared)
        m["a0"] = np.ascontiguousarray(a[b, i0:i0 + CH])
        m["s0"] = np.ascontiguousarray(s[b, i0:i0 + CH])
        m["beta0"] = np.ascontiguousarray(beta[b, i0:i0 + CH])
        m["zt"] = np.ascontiguousarray(
            z[b, i0:i0 + CH].astype(BF).transpose(2, 0, 1))
        in_maps.append(m)

    trace = bool(int(os.environ.get("KERNEL_TRACE", "0")))
    res = run_bass_kernel_spmd(nc, in_maps, core_ids=list(range(NCORES)),
                               trace=trace)
    _CACHE["res"] = res
    if trace and res.exec_time_ns is not None:
        print(f"HW exec time: {res.exec_time_ns} ns")

    out = np.zeros((B, N, D), np.float32)
    for core in range(NCORES):
        b, g = divmod(core, G)
        out[b, g * CH:(g + 1) * CH] = res.results[core]["out"]
    return out


if __name__ == "__main__":
    rng = np.random.default_rng(0)
    print("building...")
    nco = _build_nc()
    print("built ok:", len(nco.inst_map), "instructions")


# ---------------------------------------------------------------------------
# benchmarking (wall-clock steady state over PJRT; no NTFF on this axon build)
# ---------------------------------------------------------------------------

def bench(a, s, z, beta, params, iters=8):
    import time
    import jax
    import numpy as np
    from jax.sharding import Mesh, PartitionSpec
    from jax.experimental.shard_map import shard_map
    from concourse import bass2jax
    from concourse.bass2jax import _bass_exec_p, partition_id_tensor, install_neuronx_cc_hook
    import concourse.mybir as mybir

    install_neuronx_cc_hook()
    a, s, beta = _f32(a), _f32(s), _f32(beta)
    z = np.asarray(z, dtype=np.float32)
    if ("nc", False) not in _CACHE:
        _CACHE[("nc", False)] = _build_nc(debug=False)
    nc = _CACHE[("nc", False)]

    shared = _prep_shared(params)
    in_maps = []
    for core in range(NCORES):
        b, g = divmod(core, G)
        i0 = g * CH
        m = dict(shared)
        m["a0"] = np.ascontiguousarray(a[b, i0:i0 + CH])
        m["s0"] = np.ascontiguousarray(s[b, i0:i0 + CH])
        m["beta0"] = np.ascontiguousarray(beta[b, i0:i0 + CH])
        m["zt"] = np.ascontiguousarray(z[b, i0:i0 + CH].astype(BF).transpose(2, 0, 1))
        in_maps.append(m)

    partition_name = nc.partition_id_tensor.name if nc.partition_id_tensor else None
    in_names, out_names, out_avals, zero_outs = [], [], [], []
    for alloc in nc.m.functions[0].allocations:
        if not isinstance(alloc, mybir.MemoryLocationSet):
            continue
        name = alloc.memorylocations[0].name
        if alloc.kind == "ExternalInput":
            if name != partition_name:
                in_names.append(name)
        elif alloc.kind == "ExternalOutput":
            out_names.append(name)
            shape = tuple(alloc.tensor_shape)
            dtype = mybir.dt.np(alloc.dtype)
            out_avals.append(jax.core.ShapedArray(shape, dtype))
            zero_outs.append(np.zeros(shape, dtype))
    n_params = len(in_names)
    n_outs = len(out_avals)
    in_names.extend(out_names)
    if partition_name is not None:
        in_names.append(partition_name)
    donate = tuple(range(n_params, n_params + n_outs))

    def _body(*args):
        operands = list(args)
        if partition_name is not None:
            operands.append(partition_id_tensor())
        outs = _bass_exec_p.bind(
            *operands, out_avals=tuple(out_avals), in_names=tuple(in_names),
            out_names=tuple(out_names), lowering_input_output_aliases=(),
            sim_require_finite=True, sim_require_nnan=True, nc=nc)
        return tuple(outs)

    devices = jax.devices()[:NCORES]
    mesh = Mesh(np.asarray(devices), ("core",))
    sharded = jax.jit(
        shard_map(_body, mesh=mesh,
                  in_specs=(PartitionSpec("core"),) * (n_params + n_outs),
                  out_specs=(PartitionSpec("core"),) * n_outs, check_rep=False),
        donate_argnums=donate, keep_unused=True)

    per_core = [[np.asarray(m[nm]) for nm in in_names[:n_params]] for m in in_maps]
    concat_in = [np.concatenate([per_core[c][i] for c in range(NCORES)], axis=0)
                 for i in range(n_params)]
    sh_in = jax.sharding.NamedSharding(mesh, PartitionSpec("core"))
    dev_in = [jax.device_put(x, sh_in) for x in concat_in]

    def make_zeros():
        return [jax.device_put(np.zeros((NCORES * zz.shape[0], *zz.shape[1:]), zz.dtype), sh_in)
                for zz in zero_outs]

    # warmup / compile
    outs = sharded(*dev_in, *make_zeros())
    jax.block_until_ready(outs)
    times = []
    for _ in range(iters):
        zs = make_zeros()
        jax.block_until_ready(zs)
        t0 = time.perf_counter()
        outs = sharded(*dev_in, *zs)
        jax.block_until_ready(outs)
        times.append(time.perf_counter() - t0)
    times_ns = [int(t * 1e9) for t in times]
    best = min(times_ns)
    print("bench times (us):", [t // 1000 for t in times_ns])
    print(f"HW exec time: {best} ns")
    out = np.zeros((B, N, D), np.float32)
    arr = np.asarray(outs[out_names.index("out")]).reshape(NCORES, CH, D)
    for core in range(NCORES):
        b, g = divmod(core, G)
        out[b, g * CH:(g + 1) * CH] = arr[core]
    return out, best
